# revision 1
# baseline (speedup 1.0000x reference)
"""Trainium2 Bass kernel for a full transformer block (LN -> causal MHA -> residual
-> LN -> 4x MLP -> residual), SPMD across 8 NeuronCores.

Sharding: data-parallel over batch (4) x 2-way split of query rows. Each core
computes one batch element's block output for 1024 of its 2048 rows. K/V context
is computed locally over the full (front-padded) 2048-row context, so there are
no collectives. Front-padding with dummy rows puts every core's query rows at
context positions [T-TQ, T), which makes the causal structure identical across
cores (SPMD-uniform program). Pad-column contributions are excluded by zeroing
the V rows AND the softmax-denominator ones-column for pad positions (per-core
"cm" 0/1 vector), so exp needs no per-k-tile bias and can process two k-tiles
per instruction.

Matmul operands are stored in MM_DT (bf16 by default: full PE rate, half SBUF &
DMA; ~3e-3 rel err). All accumulation/LN/softmax math is fp32. Weights arrive
host-prepacked in SBUF layout so every weight DMA is contiguous per partition.
"""
import contextlib
from types import SimpleNamespace

import ml_dtypes
import numpy as np

import concourse.bass as bass
import concourse.bacc as bacc
import concourse.tile as tile
import concourse.mybir as mybir
from concourse.masks import make_identity

F32 = mybir.dt.float32
BF16 = mybir.dt.bfloat16
F32R = mybir.dt.float32r
AF = mybir.ActivationFunctionType
ALU = mybir.AluOpType
P = 128

MM_DT = BF16  # matmul operand storage dtype (BF16 or F32R)


def _np_mm_dt(mm_dt):
    return ml_dtypes.bfloat16 if mm_dt == BF16 else np.float32


def _bcast_ap(ap, parts=P):
    """[N] dram AP -> [parts, N] broadcast AP (step-0 partition dim)."""
    return bass.AP(tensor=ap.tensor, offset=ap.offset, ap=[[0, parts]] + list(ap.ap))


def _ln_tile(nc, E, pool_stats, src, dst, eps_t, tag):
    """LayerNorm one [P, C] tile: dst = (src - mu) * rstd. The per-feature
    w/b are applied later on the feature-major side (per-partition scalars)."""
    C = src.shape[-1]
    nsg = max(1, C // 512)
    sg_sz = min(512, C)
    st = pool_stats.tile([P, nsg * 6 + 3], F32, tag=f"st{tag}", name="st")
    stats = st[:, 0:nsg * 6].rearrange("p (n s) -> p n s", s=6)
    mv = st[:, nsg * 6:nsg * 6 + 2]
    rstd = st[:, nsg * 6 + 2:nsg * 6 + 3]
    for sg in range(nsg):
        nc.vector.bn_stats(out=stats[:, sg, :],
                           in_=src[:, sg * sg_sz:(sg + 1) * sg_sz])
    nc.vector.bn_aggr(out=mv, in_=stats)
    nc.scalar.activation(rstd, mv[:, 1:2], AF.Sqrt, bias=eps_t, scale=1.0)
    nc.vector.reciprocal(rstd, rstd)
    nc.vector.tensor_scalar(out=dst, in0=src, scalar1=mv[:, 0:1], scalar2=rstd,
                            op0=ALU.subtract, op1=ALU.mult)


def _transpose_tile(nc, E, G, psum_pool, src, dstT, col0, tag):
    """Transpose [P, C] token-major tile into feature-major dstT[:, :, col0:+P],
    batching TG 128x128 transposes per PSUM bank to cut copyback count."""
    TG = min(4, E.CCH)
    for cc0 in range(0, E.CCH, TG):
        pt = psum_pool.tile([P, TG, P], src.dtype, tag=tag, name="pt")
        for j in range(TG):
            nc.tensor.transpose(pt[:, j, :],
                                src[:, (cc0 + j) * P:(cc0 + j + 1) * P], G.ident)
        nc.vector.tensor_copy(dstT[:, cc0:cc0 + TG, col0:col0 + P], pt)


def _phase_a(nc, tc, E, G):
    """LN1 over the full context + transpose to feature-major hT."""
    with tc.tile_pool(name="lna", bufs=3) as lna, \
         tc.tile_pool(name="lnst", bufs=4) as lnst, \
         tc.tile_pool(name="tpsA", bufs=3, space="PSUM") as tpsA:
        for rt in range(E.NT):
            x_t = lna.tile([P, E.C], F32, tag="x", name="x_t")
            nc.sync.dma_start(x_t, G.x[rt * P:(rt + 1) * P, :])
            h_t = lna.tile([P, E.C], E.MMDT, tag="h", name="h_t")
            _ln_tile(nc, E, lnst, x_t, h_t, G.eps_t, "a")
            _transpose_tile(nc, E, G, tpsA, h_t, G.hT, rt * P, "tp")


def _attn_pair(nc, E, G, A, g, vg):
    """Attention for head pair g (heads 2g, 2g+1) — QK row-packed, exp paired
    over two k-tiles."""
    lc0 = 2 * g - vg * E.VH
    cc = g
    for qc in range(E.NQC):
        nkt = (E.PAD_Q + (qc + 1) * E.QC) // P
        assert nkt % 2 == 0
        nkp = nkt // 2
        yp = [A.ypsp.tile([E.D + 1, E.QC], F32, tag=f"y{hh}", name="yp")
              for hh in (0, 1)]
        for ktp in range(nkp):
            kt0 = 2 * ktp
            sp = [A.spsp.tile([P, 2, E.QC], F32, tag=f"s{hh}", name="sp")
                  for hh in (0, 1)]
            for j in (0, 1):
                for hh in (0, 1):
                    poff = hh * E.D
                    nc.tensor.matmul(
                        sp[hh][:, j, :],
                        A.kTg[poff:poff + E.D, (kt0 + j) * P:(kt0 + j + 1) * P],
                        A.qTg[poff:poff + E.D, qc * E.QC:(qc + 1) * E.QC],
                        start=True, stop=True)
            for j in (0, 1):
                d = (kt0 + j) * P - E.PAD_Q - qc * E.QC
                if -P < d < E.QC:
                    w = min(E.QC, P + d)
                    for hh in (0, 1):
                        nc.vector.tensor_tensor(
                            sp[hh][:, j, :w], sp[hh][:, j, :w],
                            G.tri[:, d // P, :w], ALU.add)
            for hh in (0, 1):
                p_t = A.attp.tile([P, 2, E.QC], E.MMDT, tag=f"p{hh}", name="p_t")
                nc.scalar.activation(p_t, sp[hh], AF.Exp, scale=E.SD)
                for j in (0, 1):
                    nc.tensor.matmul(
                        yp[hh], A.vTok[:, kt0 + j, lc0 + hh, :], p_t[:, j, :],
                        start=(ktp == 0 and j == 0),
                        stop=(ktp == nkp - 1 and j == 1))
        for hh in (0, 1):
            poff = hh * E.D
            nrm = A.attp.tile([1, E.QC], F32, tag="nrm", name="nrm")
            nc.vector.reciprocal(nrm, yp[hh][E.D:E.D + 1, :])
            nbc = A.attp.tile([E.D, E.QC], F32, tag="nbc", name="nbc")
            nc.gpsimd.partition_broadcast(nbc, nrm)
            if poff == 0:
                nc.vector.tensor_tensor(
                    G.yT[0:E.D, cc, qc * E.QC:(qc + 1) * E.QC],
                    yp[hh][0:E.D, :], nbc, ALU.mult)
            else:
                tmp = A.attp.tile([E.D, E.QC], E.MMDT, tag="ytmp", name="tmp")
                nc.vector.tensor_tensor(tmp, yp[hh][0:E.D, :], nbc, ALU.mult)
                nc.sync.dma_start(
                    G.yT[E.D:2 * E.D, cc, qc * E.QC:(qc + 1) * E.QC], tmp)


def _phase_bc(nc, tc, E, G):
    """QKV projections + attention, in V-groups of VH heads."""
    with contextlib.ExitStack() as st:
        A = SimpleNamespace()
        wkqp = st.enter_context(tc.tile_pool(name="wkq", bufs=2))
        wvp = st.enter_context(tc.tile_pool(name="wv", bufs=2))
        kqgp = st.enter_context(tc.tile_pool(name="kqg", bufs=1))
        vtokp = st.enter_context(tc.tile_pool(name="vtok", bufs=1))
        A.attp = st.enter_context(tc.tile_pool(name="att", bufs=3))
        prjp = st.enter_context(tc.tile_pool(name="prj", bufs=1, space="PSUM"))
        vprjp = st.enter_context(tc.tile_pool(name="vprj", bufs=1, space="PSUM"))
        A.spsp = st.enter_context(tc.tile_pool(name="sps", bufs=1, space="PSUM"))
        A.ypsp = st.enter_context(tc.tile_pool(name="yps", bufs=1, space="PSUM"))

        for vg in range(E.NVG):
            # ---- V projection for VH heads (token-major, with ones column).
            # Pad rows (cm==0) are zeroed in both V and the ones column, which
            # removes them from the softmax numerator AND denominator. ----
            VW = E.VH * E.D
            wv_t = wvp.tile([P, E.CCH, VW], E.MMDT, tag="wv", name="wv_t")
            nc.sync.dma_start(wv_t, G.Wv4[:, vg])
            A.vTok = vtokp.tile([P, E.NT, E.VH, E.D + 1], E.MMDT, tag="vt",
                                name="vTok")
            for kt in range(E.NT):
                vps = vprjp.tile([P, VW], F32, tag="vp", name="vps")
                for ci in range(E.CCH):
                    nc.tensor.matmul(vps, G.hT[:, ci, kt * P:(kt + 1) * P],
                                     wv_t[:, ci, :],
                                     start=(ci == 0), stop=(ci == E.CCH - 1))
                nc.vector.tensor_tensor(vps, vps,
                                        G.bv_b[:, vg * VW:(vg + 1) * VW], ALU.add)
                nc.vector.tensor_scalar_mul(
                    A.vTok[:, kt, :, 0:E.D],
                    vps.rearrange("p (h d) -> p h d", d=E.D),
                    scalar1=G.cm_s[:, kt:kt + 1])
                nc.vector.tensor_scalar_mul(
                    A.vTok[:, kt, :, E.D:E.D + 1], G.ones4[:, :, None],
                    scalar1=G.cm_s[:, kt:kt + 1])

            for sub in range(E.VH // 2):
                g = vg * (E.VH // 2) + sub
                # ---- K projection (feature-major) ----
                wk_t = wkqp.tile([P, E.CCH, P], E.MMDT, tag="wk", name="wk_t")
                nc.sync.dma_start(wk_t, G.Wk4[:, g])
                A.kTg = kqgp.tile([P, E.T], E.MMDT, tag="k", name="kTg")
                for tcn in range(E.NTC):
                    kps = prjp.tile([P, E.TC], F32, tag="pp", name="kps")
                    for ci in range(E.CCH):
                        nc.tensor.matmul(
                            kps, wk_t[:, ci, :],
                            G.hT[:, ci, tcn * E.TC:(tcn + 1) * E.TC],
                            start=(ci == 0), stop=(ci == E.CCH - 1))
                    nc.scalar.activation(A.kTg[:, tcn * E.TC:(tcn + 1) * E.TC],
                                         kps, AF.Identity,
                                         bias=G.bk_s[:, g:g + 1], scale=1.0)
                # ---- Q projection (feature-major, query rows only) ----
                wq_t = wkqp.tile([P, E.CCH, P], E.MMDT, tag="wq", name="wq_t")
                nc.sync.dma_start(wq_t, G.Wq4[:, g])
                A.qTg = kqgp.tile([P, E.TQ], E.MMDT, tag="q", name="qTg")
                for tcn in range(E.NQTC):
                    qps = prjp.tile([P, E.QTC], F32, tag="pp", name="qps")
                    for ci in range(E.CCH):
                        nc.tensor.matmul(
                            qps, wq_t[:, ci, :],
                            G.hT[:, ci,
                                 E.PAD_Q + tcn * E.QTC:E.PAD_Q + (tcn + 1) * E.QTC],
                            start=(ci == 0), stop=(ci == E.CCH - 1))
                    nc.scalar.activation(A.qTg[:, tcn * E.QTC:(tcn + 1) * E.QTC],
                                         qps, AF.Identity,
                                         bias=G.bq_s[:, g:g + 1], scale=1.0)
                _attn_pair(nc, E, G, A, g, vg)


def _phase_d(nc, tc, E, G):
    """O-projection + residual + LN2 + transpose to h2T; x2+b2 -> dram scratch."""
    with tc.tile_pool(name="wo", bufs=1) as wop, \
         tc.tile_pool(name="dwork", bufs=2) as dwork, \
         tc.tile_pool(name="lnst2", bufs=4) as lnst2, \
         tc.tile_pool(name="ops", bufs=2, space="PSUM") as opsp, \
         tc.tile_pool(name="tpsD", bufs=2, space="PSUM") as tpsD:
        wo_t = wop.tile([P, E.CCH, E.C], E.MMDT, name="wo_t")
        nc.sync.dma_start(wo_t, G.Wo3)
        for tt in range(E.NQT):
            xr_t = dwork.tile([P, E.C], F32, tag="xr", name="xr_t")
            nc.sync.dma_start(xr_t, G.x[E.PAD_Q + tt * P:E.PAD_Q + (tt + 1) * P, :])
            x2_t = dwork.tile([P, E.C], F32, tag="x2", name="x2_t")
            for oc in range(E.NOC):
                ops = opsp.tile([P, E.OC], F32, tag="op", name="ops")
                for ci in range(E.CCH):
                    nc.tensor.matmul(ops, G.yT[:, ci, tt * P:(tt + 1) * P],
                                     wo_t[:, ci, oc * E.OC:(oc + 1) * E.OC],
                                     start=(ci == 0), stop=(ci == E.CCH - 1))
                nc.vector.tensor_tensor(x2_t[:, oc * E.OC:(oc + 1) * E.OC], ops,
                                        G.bo_b[:, oc * E.OC:(oc + 1) * E.OC],
                                        ALU.add)
            nc.vector.tensor_tensor(x2_t, x2_t, xr_t, ALU.add)
            nc.vector.tensor_tensor(G.x2b_sb[:, tt, :], x2_t, G.b2_b, ALU.add)
            h2_t = dwork.tile([P, E.C], E.MMDT, tag="h2", name="h2_t")
            _ln_tile(nc, E, lnst2, x2_t, h2_t, G.eps_t, "d")
            _transpose_tile(nc, E, G, tpsD, h2_t, G.h2T, tt * P, "tp")


def _phase_e(nc, tc, E, G):
    """MLP: u = relu(h2 @ W1 + b1); y_acc = u @ W2, sliced over F.

    u for a whole (f-slice, token-chunk) is staged in SBUF so both the u and y
    matmuls run at N=TQC (512) with only 2+2 PSUM banks live."""
    with tc.tile_pool(name="w1", bufs=2) as w1p, \
         tc.tile_pool(name="w2", bufs=2) as w2p, \
         tc.tile_pool(name="uall", bufs=2) as uallp, \
         tc.tile_pool(name="ups", bufs=2, space="PSUM") as upsp, \
         tc.tile_pool(name="ypsE", bufs=2, space="PSUM") as ypsEp:
        for fs in range(E.NFS):
            w1_t = w1p.tile([P, E.CCH, E.FS], E.MMDT, tag="w1", name="w1_t")
            nc.sync.dma_start(w1_t, G.W14[:, fs])
            w2_t = w2p.tile([P, E.NFC, E.C], E.MMDT, tag="w2", name="w2_t")
            nc.sync.dma_start(w2_t, G.W24[:, fs])
            for tq in range(E.NTQC):
                u_all = uallp.tile([P, E.NFC, E.TQC], E.MMDT, tag="ua",
                                   name="u_all")
                for fc in range(E.NFC):
                    ups = upsp.tile([P, E.TQC], F32, tag="u", name="ups")
                    for ci in range(E.CCH):
                        nc.tensor.matmul(
                            ups, w1_t[:, ci, fc * P:(fc + 1) * P],
                            G.h2T[:, ci, tq * E.TQC:(tq + 1) * E.TQC],
                            start=(ci == 0), stop=(ci == E.CCH - 1))
                    fi = fs * E.NFC + fc
                    nc.scalar.activation(u_all[:, fc, :], ups, AF.Relu,
                                         bias=G.b1_s[:, fi:fi + 1], scale=1.0)
                for t2 in range(E.TSUB):
                    tt = tq * E.TSUB + t2
                    for oc in range(E.NOC):
                        yps = ypsEp.tile([P, E.OC], F32, tag="y", name="yps")
                        for fc in range(E.NFC):
                            nc.tensor.matmul(
                                yps, u_all[:, fc, t2 * P:(t2 + 1) * P],
                                w2_t[:, fc, oc * E.OC:(oc + 1) * E.OC],
                                start=(fc == 0), stop=(fc == E.NFC - 1))
                        dst = G.y_acc[:, tt, oc * E.OC:(oc + 1) * E.OC]
                        if fs == 0:
                            nc.vector.tensor_copy(dst, yps)
                        else:
                            nc.vector.tensor_tensor(dst, dst, yps, ALU.add)


def _trace_main(nc, tc, E, G):
    """One full block computation (phases A-E + final)."""
    import concourse.tile as tile  # noqa: F401
    P_ = P
    with tc.tile_pool(name="h2T", bufs=1) as h2Tp:
        with tc.tile_pool(name="yT", bufs=1) as yTp:
            with tc.tile_pool(name="hT", bufs=1) as hTp:
                G.hT = hTp.tile([P_, E.CCH, E.T], E.MMDT, name="hT")
                _phase_a(nc, tc, E, G)
                G.yT = yTp.tile([P_, E.CCH, E.TQ], E.MMDT, name="yT")
                _phase_bc(nc, tc, E, G)
            G.h2T = h2Tp.tile([P_, E.CCH, E.TQ], E.MMDT, name="h2T")
            G.x2b_sb = h2Tp.tile([P_, E.NQT, E.C], F32, tag="x2b", name="x2b_sb")
            _phase_d(nc, tc, E, G)

        with tc.tile_pool(name="yacc", bufs=1) as yaccp:
            G.y_acc = yaccp.tile([P_, E.NQT, E.C], F32, name="y_acc")
            _phase_e(nc, tc, E, G)

            # ---------- final: out = y_acc + (x2 + b2) ----------
            with tc.tile_pool(name="fin", bufs=3) as finp:
                for tt in range(E.NQT):
                    o_t = finp.tile([P_, E.C], F32, tag="o", name="o_t")
                    nc.vector.tensor_tensor(o_t, G.y_acc[:, tt, :],
                                            G.x2b_sb[:, tt, :], ALU.add)
                    nc.sync.dma_start(G.out[tt * P_:(tt + 1) * P_, :], o_t)


def build_nc(T=2048, TQ=1024, C=1024, H=16, D=64, F=4096, n_cores=8, mm_dt=MM_DT,
             body_reps=1):
    assert C == H * D and C % P == 0 and T % P == 0 and TQ % P == 0 and F % P == 0
    E = SimpleNamespace(T=T, TQ=TQ, C=C, H=H, D=D, F=F, MMDT=mm_dt)
    E.CCH = C // P
    E.NT = T // P
    E.NQT = TQ // P
    E.PAD_Q = T - TQ
    E.QC = min(512, TQ)
    E.NQC = TQ // E.QC
    E.TC = min(512, T)
    E.NTC = T // E.TC
    E.QTC = min(512, TQ)
    E.NQTC = TQ // E.QTC
    E.OC = min(512, C)
    E.NOC = C // E.OC
    E.VH = min(8, H)
    E.NVG = H // E.VH
    E.NPairs = H // 2
    E.FS = min(1024, F)
    E.NFS = F // E.FS
    E.NFC = E.FS // P
    E.TQC = min(512, TQ)
    E.NTQC = TQ // E.TQC
    E.TSUB = E.TQC // P
    E.SD = float(1.0 / np.sqrt(D))

    nc = bacc.Bacc("TRN2", target_bir_lowering=False, debug=False,
                   num_devices=n_cores)
    G = SimpleNamespace()
    G.x = nc.dram_tensor("x", [T, C], F32, kind="ExternalInput").ap()
    G.cm = nc.dram_tensor("cm", [T], F32, kind="ExternalInput").ap()
    # host-prepacked weights (contiguous per-partition SBUF layouts)
    VW = E.VH * E.D
    G.Wq4 = nc.dram_tensor("Wq", [P, E.NPairs, E.CCH, P], mm_dt,
                           kind="ExternalInput").ap()
    G.Wk4 = nc.dram_tensor("Wk", [P, E.NPairs, E.CCH, P], mm_dt,
                           kind="ExternalInput").ap()
    G.Wv4 = nc.dram_tensor("Wv", [P, E.NVG, E.CCH, VW], mm_dt,
                           kind="ExternalInput").ap()
    G.Wo3 = nc.dram_tensor("Wo", [P, E.CCH, C], mm_dt, kind="ExternalInput").ap()
    G.W14 = nc.dram_tensor("W1", [P, E.NFS, E.CCH, E.FS], mm_dt,
                           kind="ExternalInput").ap()
    G.W24 = nc.dram_tensor("W2", [P, E.NFS, E.NFC, C], mm_dt,
                           kind="ExternalInput").ap()
    vecs = {}
    for nm in ("bq", "bk", "bv", "bo", "b2"):
        vecs[nm] = nc.dram_tensor(nm, [C], F32, kind="ExternalInput").ap()
    vecs["b1"] = nc.dram_tensor("b1", [F], F32, kind="ExternalInput").ap()
    G.out = nc.dram_tensor("out", [TQ, C], F32, kind="ExternalOutput").ap()

    with tile.TileContext(nc) as tc, contextlib.ExitStack() as ctx:
        dram = ctx.enter_context(tc.tile_pool(name="dram", bufs=1, space="DRAM"))
        glob = ctx.enter_context(tc.tile_pool(name="glob", bufs=1))

        G.ident = glob.tile([P, P], E.MMDT, name="ident")
        make_identity(nc, G.ident)
        # packed small per-partition vectors
        nsm = 1 + E.VH + 2 * E.CCH + F // P + E.NT
        sm = glob.tile([P, nsm], F32, name="sm")
        o = 0
        G.eps_t = sm[:, o:o + 1]; o += 1
        G.ones4 = sm[:, o:o + E.VH]; o += E.VH
        G.bq_s = sm[:, o:o + E.CCH]; o += E.CCH
        G.bk_s = sm[:, o:o + E.CCH]; o += E.CCH
        G.b1_s = sm[:, o:o + F // P]; o += F // P
        G.cm_s = sm[:, o:o + E.NT]; o += E.NT
        nc.vector.memset(G.eps_t, 1e-5)
        nc.vector.memset(G.ones4, 1.0)
        nc.sync.dma_start(G.bq_s, vecs["bq"].rearrange("(o p) -> p o", p=P))
        nc.sync.dma_start(G.bk_s, vecs["bk"].rearrange("(o p) -> p o", p=P))
        nc.sync.dma_start(G.b1_s, vecs["b1"].rearrange("(o p) -> p o", p=P))
        nc.sync.dma_start(G.cm_s, G.cm.rearrange("(kt p) -> p kt", p=P))

        for nm, src in (("bo_b", vecs["bo"]), ("b2_b", vecs["b2"]),
                        ("bv_b", vecs["bv"])):
            t_ = glob.tile([P, C], F32, tag=nm, name=nm)
            nc.gpsimd.dma_start(out=t_, in_=_bcast_ap(src))
            setattr(G, nm, t_)

        # straddle masks packed: tri[:, i, r] = -1e9 where r < p + i*128
        G.tri = glob.tile([P, E.QC // P, E.QC], F32, name="tri")
        nc.gpsimd.memset(G.tri, 0.0)
        for i in range(E.QC // P):
            nc.gpsimd.affine_select(
                out=G.tri[:, i, :], in_=G.tri[:, i, :], compare_op=ALU.is_ge,
                fill=-1e9, base=-i * P, pattern=[[1, E.QC]],
                channel_multiplier=-1)

        for _rep in range(body_reps):
            _trace_main(nc, tc, E, G)

    nc.compile()
    return nc


# ----------------------------------------------------------------------------
# Host entry point: takes FULL inputs, shards, runs 8 cores, gathers.
# ----------------------------------------------------------------------------
_NC_CACHE = {}


def _get_nc():
    if "full" not in _NC_CACHE:
        _NC_CACHE["full"] = build_nc()
    return _NC_CACHE["full"]


def _pack_weights(inputs, T, TQ, C, H, D, F, mm_dt=MM_DT):
    """Prepack weights into contiguous per-partition SBUF layouts."""
    wdt = _np_mm_dt(mm_dt)
    CCH = C // P
    NPairs = H // 2
    VH = min(8, H)
    NVG = H // VH
    VW = VH * D
    FS = min(1024, F)
    NFS = F // FS
    NFC = FS // P

    ln1w = np.asarray(inputs["ln1_w"], dtype=np.float32)
    ln2w = np.asarray(inputs["ln2_w"], dtype=np.float32)

    def w(k):
        a = np.asarray(inputs[k], dtype=np.float32)
        if k in ("Wq", "Wk", "Wv"):
            a = a * ln1w[:, None]   # fold LN1 scale
        elif k == "W1":
            a = a * ln2w[:, None]   # fold LN2 scale
        return a.astype(wdt)

    out = {}
    # [C_in, M] -> [p, group, ko, m]
    out["Wq"] = np.ascontiguousarray(
        w("Wq").reshape(CCH, P, NPairs, P).transpose(1, 2, 0, 3))
    out["Wk"] = np.ascontiguousarray(
        w("Wk").reshape(CCH, P, NPairs, P).transpose(1, 2, 0, 3))
    out["Wv"] = np.ascontiguousarray(
        w("Wv").reshape(CCH, P, NVG, VW).transpose(1, 2, 0, 3))
    out["Wo"] = np.ascontiguousarray(
        w("Wo").reshape(CCH, P, C).transpose(1, 0, 2))
    out["W1"] = np.ascontiguousarray(
        w("W1").reshape(CCH, P, NFS, FS).transpose(1, 2, 0, 3))
    out["W2"] = np.ascontiguousarray(
        w("W2").reshape(NFS, NFC, P, C).transpose(2, 0, 1, 3))
    return out


def prepare_common(inputs, T, TQ, C, H, D, F, mm_dt=MM_DT):
    f32 = lambda k: np.asarray(inputs[k], dtype=np.float32)
    ln1b, ln2b = f32("ln1_b"), f32("ln2_b")
    common = {
        # fold LN1/LN2 bias through the following matmul into its bias
        "bq": np.ascontiguousarray(ln1b @ f32("Wq") + f32("bq")),
        "bk": np.ascontiguousarray(ln1b @ f32("Wk") + f32("bk")),
        "bv": np.ascontiguousarray(ln1b @ f32("Wv") + f32("bv")),
        "b1": np.ascontiguousarray(ln2b @ f32("W1") + f32("b1")),
        "bo": np.ascontiguousarray(f32("bo")),
        "b2": np.ascontiguousarray(f32("b2")),
    }
    common.update(_pack_weights(inputs, T, TQ, C, H, D, F, mm_dt))
    return common


def make_in_maps(inputs, n_cores=8, mm_dt=MM_DT):
    x = np.asarray(inputs["x"], dtype=np.float32)
    B, T, C = x.shape
    TQ = (B * T) // n_cores
    H, D, F = 16, 64, 4096
    common = prepare_common(inputs, T, TQ, C, H, D, F, mm_dt)
    in_maps = []
    for c in range(n_cores):
        b = c // 2
        off = (c % 2) * TQ
        pad = T - (off + TQ)
        x_ctx = np.zeros((T, C), dtype=np.float32)
        x_ctx[pad:, :] = x[b, :off + TQ, :]
        cmv = (np.arange(T) >= pad).astype(np.float32)  # 0 on pad rows, else 1
        in_maps.append({"x": x_ctx, "cm": cmv, **common})
    return in_maps


def kernel(**inputs):
    from concourse.bass_utils import run_bass_kernel_spmd

    x = np.asarray(inputs["x"], dtype=np.float32)
    B, T, C = x.shape          # (4, 2048, 1024)
    n_cores = 8
    TQ = (B * T) // n_cores    # 1024 query rows per core

    nc = _get_nc()
    in_maps = make_in_maps(inputs, n_cores)
    res = run_bass_kernel_spmd(nc, in_maps, core_ids=list(range(n_cores)),
                               trace=False)

    out = np.empty((B, T, C), dtype=np.float32)
    for c in range(n_cores):
        b = c // 2
        off = (c % 2) * TQ
        out[b, off:off + TQ, :] = res.results[c]["out"]
    return out

